# revision 10
# baseline (speedup 1.0000x reference)
"""Distributed 2-layer GCN (PyG GCNConv-style) on 8 Trainium2 NeuronCores.

Strategy (matches the sharding hint):
  - Nodes are sharded 2500/core (padded to 2560 = 20 blocks of 128).
  - Edges are partitioned by destination node; per (core, dst-block) the
    incoming edges (+ self loops) are packed into uniform KT tiles of 128
    edge slots. The sym-normalization coefficients are precomputed on the
    host into per-block selection matrices S [128 edge-slots, 128 dst] so
    scatter-add becomes a TensorE matmul.
  - Layer 1 aggregates in INPUT space (A @ x, width F=512) before the W1
    matmul; layer 2 aggregates AFTER the W2 projection (width 128). Both
    minimize gathered bytes. The gathered x and the S coefficients are fp8
    (e4m3) with DoubleRow matmuls: halves gather HBM traffic and doubles
    the aggregation matmul rate (rel err ~1.2e-2 vs 3.2e-3 all-bf16).
  - h@rw2 + b2 is computed in phase A (while h is in SBUF), so phase B
    only needs the gathered hw rows: no h table is kept.
  - The halo exchange (AllGather of the hw = h@W2 table, fp8 = 2.6 MB) is
    CHUNKED: one AllGather per 5 dst-blocks, issued as soon as those hw
    blocks are written, so the collective overlaps the rest of phase A.
  - Weights are replicated; everything is laid out feature-major on the
    host so the kernel needs zero on-device transposes.

kernel(**inputs) takes the FULL unsharded inputs and returns the FULL
[20000, 128] float32 output.
"""

import math

import numpy as np
import ml_dtypes

import concourse.bass as bass
import concourse.mybir as mybir
import concourse.tile as tile
from concourse import bacc
from concourse.bass_utils import run_bass_kernel_spmd

# ----------------------------------------------------------------------------
# configuration
# ----------------------------------------------------------------------------
C = 8          # cores
FP8_L1 = True  # gather x + S1 in fp8 e4m3, DoubleRow aggregation matmuls
FP8_HW = False  # hw table fp8 e4m3 (collective + phase-B gather), upconvert
CHUNKS = 4     # AllGather chunks (must divide B)
GB = 2         # dst-blocks per dma_gather op (GB*KT*128 <= 4096 ring)

_DT = {
    "bf16": (mybir.dt.bfloat16, ml_dtypes.bfloat16),
    "f32": (mybir.dt.float32, np.float32),
    "fp8": (mybir.dt.float8e4, ml_dtypes.float8_e4m3),
}
COMPUTE = "bf16"

_prog_cache: dict = {}


def _cfg_from_shapes(x, w1, w2):
    n, f = x.shape
    h = w1.shape[1]
    out = w2.shape[1]
    assert n % C == 0, n
    nl = n // C                      # real nodes per core
    nlp = ((nl + 127) // 128) * 128  # padded nodes per core
    b = nlp // 128                   # dst blocks per core
    assert f % 128 == 0 and h % 128 == 0 and out % 128 == 0
    return dict(N=n, F=f, H=h, OUT=out, NL=nl, NLP=nlp, B=b, NP=C * nlp,
                FK=f // 128, HC=h // 128, OC=out // 128)


# ----------------------------------------------------------------------------
# host-side preprocessing: graph partition + norm coefficients + layouts
# ----------------------------------------------------------------------------
def _preprocess(x, edge_index, edge_weight, w1, b1, w2, b2, rw1, rb1, rw2, rb2,
                cfg, np_cdt):
    N, F, H, OUT = cfg["N"], cfg["F"], cfg["H"], cfg["OUT"]
    NL, NLP, B, NP = cfg["NL"], cfg["NLP"], cfg["B"], cfg["NP"]
    HC, FK = cfg["HC"], cfg["FK"]
    np_l1 = ml_dtypes.float8_e4m3 if FP8_L1 else np_cdt

    row = np.asarray(edge_index[0], dtype=np.int64)
    col = np.asarray(edge_index[1], dtype=np.int64)
    ew = np.asarray(edge_weight, dtype=np.float32)

    # symmetric normalization, exactly like the reference (self loop weight 1)
    deg = np.bincount(col, weights=ew.astype(np.float64), minlength=N) + 1.0
    deg = deg.astype(np.float32)
    dis = np.where(deg > 0, 1.0 / np.sqrt(np.where(deg > 0, deg, 1.0)), 0.0)
    dis = dis.astype(np.float32)

    loop = np.arange(N, dtype=np.int64)
    srcs = np.concatenate([row, loop])
    dsts = np.concatenate([col, loop])
    norms = np.concatenate([dis[row] * ew * dis[col], dis * dis])

    # padded node ids: node g lives on core g//NL at local slot g%NL
    src_pad = (srcs // NL) * NLP + (srcs % NL)
    core = dsts // NL
    local = dsts % NL
    blk = local // 128
    dloc = local % 128

    key = (core * B + blk).astype(np.int64)
    order = np.argsort(key, kind="stable")
    key_s = key[order]
    counts = np.bincount(key_s, minlength=C * B)
    starts = np.zeros(C * B, dtype=np.int64)
    np.cumsum(counts[:-1], out=starts[1:])
    pos = np.arange(key_s.size, dtype=np.int64) - starts[key_s]

    KT = max(1, int(math.ceil(counts.max() / 128)))
    if KT % 2 and FP8_L1:
        KT += 1  # even KT for DoubleRow pairs

    src_s = src_pad[order].astype(np.int32)
    core_s = core[order]
    blk_s = blk[order]
    dloc_s = dloc[order]
    norm_s = norms[order]
    kt_s = pos // 128
    p_s = pos % 128

    # int16 indices for dma_gather: slot i of block b -> [i%16, b*KT*8 + i//16],
    # replicated across the 8 groups of 16 partitions
    assert NP < 2 ** 15
    slot = kt_s * 128 + p_s
    idx16 = np.zeros((C, 16, B * KT * 8), dtype=np.int16)
    idx16[core_s, slot % 16, blk_s * (KT * 8) + slot // 16] = \
        src_s.astype(np.int16)
    idx16_all = np.tile(idx16, (1, 8, 1))

    # phase-B indices: hw table is CHUNK-major (each chunked AllGather
    # writes a contiguous [C*BC*128, OUT] region):
    #   row(src) = g*C*BC*128 + core*BC*128 + (lp % (BC*128)), g = lp//(BC*128)
    nchunk = CHUNKS if B % CHUNKS == 0 else 1
    BCB = (B // nchunk) * 128
    src_core = srcs // NL
    src_lp = srcs % NL
    row_b = ((src_lp // BCB) * C * BCB + src_core * BCB + src_lp % BCB)
    row_b_s = row_b[order].astype(np.int16)
    idx16b = np.zeros((C, 16, B * KT * 8), dtype=np.int16)
    idx16b[core_s, slot % 16, blk_s * (KT * 8) + slot // 16] = row_b_s
    idx16b_all = np.tile(idx16b, (1, 8, 1))

    # S coefficients, partition(slot)-major: S[c, p, b, kt, d]
    S_all = np.zeros((C, 128, B, KT, 128), dtype=np.float32)
    S_all[core_s, p_s, blk_s, kt_s, dloc_s] = norm_s
    s1_all = S_all.astype(np_l1)
    s2_all = S_all.astype(np_cdt)

    # padded, replicated x table [NP, F] (fp8 when FP8_L1)
    x = np.asarray(x, dtype=np.float32)
    x_table = np.zeros((NP, F), dtype=np.float32)
    x_table.reshape(C, NLP, F)[:, :NL] = x.reshape(C, NL, F)
    x_table = x_table.astype(np_l1)

    # feature-major x per core (bf16): xT[p, k, n] = x_core[n, k*128+p]
    xT_all = np.ascontiguousarray(
        np.zeros((C, NLP, F), np.float32).reshape(C, NLP, FK, 128)
        .transpose(0, 3, 2, 1)).astype(np_cdt)
    xt_f32 = np.zeros((C, NLP, F), dtype=np.float32)
    xt_f32[:, :NL] = x.reshape(C, NL, F)
    xT_all = np.ascontiguousarray(
        xt_f32.reshape(C, NLP, FK, 128).transpose(0, 3, 2, 1)).astype(np_cdt)

    w1 = np.asarray(w1, np.float32)
    rw1 = np.asarray(rw1, np.float32)
    w2 = np.asarray(w2, np.float32)
    rw2 = np.asarray(rw2, np.float32)
    b1c = (np.asarray(b1, np.float32) + np.asarray(rb1, np.float32))
    b2c = (np.asarray(b2, np.float32) + np.asarray(rb2, np.float32))

    # [128, FK, H] : w1_in[p, k, j] = w1[k*128+p, j]
    w1_in = np.ascontiguousarray(
        w1.reshape(FK, 128, H).transpose(1, 0, 2)).astype(np_cdt)
    rw1_in = np.ascontiguousarray(
        rw1.reshape(FK, 128, H).transpose(1, 0, 2)).astype(np_cdt)
    w2_in = np.ascontiguousarray(
        w2.reshape(HC, 128, OUT).transpose(1, 0, 2)).astype(np_cdt)
    rw2b_in = np.zeros((128, HC + 1, OUT), dtype=np.float32)
    rw2b_in[:, :HC] = rw2.reshape(HC, 128, OUT).transpose(1, 0, 2)
    rw2b_in[0, HC, :] = b2c
    rw2b_in = rw2b_in.astype(np_cdt)

    bias1_in = np.ascontiguousarray(b1c.reshape(HC, 128).T).astype(np.float32)

    in_maps = []
    for c in range(C):
        in_maps.append({
            "x_table": x_table,
            "idx16_in": np.ascontiguousarray(idx16_all[c]),
            "idx16b_in": np.ascontiguousarray(idx16b_all[c]),
            "s1_in": np.ascontiguousarray(
                s1_all[c].reshape(128, B, KT * 128)),
            "s2_in": np.ascontiguousarray(
                s2_all[c].reshape(128, B, KT * 128)),
            "xt_in": np.ascontiguousarray(xT_all[c]),
            "w1_in": w1_in,
            "rw1_in": rw1_in,
            "w2_in": w2_in,
            "rw2b_in": rw2b_in,
            "bias1_in": bias1_in,
        })
    return in_maps, KT


# ----------------------------------------------------------------------------
# device program
# ----------------------------------------------------------------------------
def _build(cfg, KT, cdt, debug_out=False, reps=1, no_collective=False):
    F, H, OUT = cfg["F"], cfg["H"], cfg["OUT"]
    NLP, B, NP = cfg["NLP"], cfg["B"], cfg["NP"]
    FK, HC, OC = cfg["FK"], cfg["HC"], cfg["OC"]
    f32 = mybir.dt.float32
    l1dt = mybir.dt.float8e4 if FP8_L1 else cdt
    hwdt = mybir.dt.float8e4 if FP8_HW else cdt
    nchunk = CHUNKS if B % CHUNKS == 0 else 1
    BC = B // nchunk          # blocks per collective chunk
    assert GB * KT * 128 <= 4096

    nc = bacc.Bacc("TRN2", target_bir_lowering=False, debug=False,
                   enable_asserts=False, num_devices=C,
                   dynamic_dma_scratch_size=65536, num_swdge_queues=2)

    x_table = nc.dram_tensor("x_table", [NP, F], l1dt, kind="ExternalInput")
    idx16_in = nc.dram_tensor("idx16_in", [128, B * KT * 8], mybir.dt.int16,
                              kind="ExternalInput")
    idx16b_in = nc.dram_tensor("idx16b_in", [128, B * KT * 8], mybir.dt.int16,
                               kind="ExternalInput")
    s1_in = nc.dram_tensor("s1_in", [128, B, KT * 128], l1dt,
                           kind="ExternalInput")
    s2_in = nc.dram_tensor("s2_in", [128, B, KT * 128], cdt,
                           kind="ExternalInput")
    xt_in = nc.dram_tensor("xt_in", [128, FK, NLP], cdt, kind="ExternalInput")
    w1_in = nc.dram_tensor("w1_in", [128, FK, H], cdt, kind="ExternalInput")
    rw1_in = nc.dram_tensor("rw1_in", [128, FK, H], cdt, kind="ExternalInput")
    w2_in = nc.dram_tensor("w2_in", [128, HC, OUT], cdt, kind="ExternalInput")
    rw2b_in = nc.dram_tensor("rw2b_in", [128, HC + 1, OUT], cdt,
                             kind="ExternalInput")
    bias1_in = nc.dram_tensor("bias1_in", [128, HC], f32,
                              kind="ExternalInput")
    out_d = nc.dram_tensor("out", [NLP, OUT], f32, kind="ExternalOutput")

    def _gather(out_tile, table, idx16_sb, b0, nblk, elem):
        # one op per GB blocks; KT*128 slots per block
        nidx = nblk * KT * 128
        nc.gpsimd.dma_gather(
            out_ap=out_tile[:, 0:nblk * KT], in_ap=table[:],
            idxs_ap=idx16_sb[:, b0 * KT * 8:(b0 + nblk) * KT * 8],
            num_idxs=nidx, num_idxs_reg=nidx,
            elem_size=elem, single_packet=False,
            queue_num=(b0 // GB) % 2)

    with tile.TileContext(nc) as tc:
        with (
            tc.tile_pool(name="dram", bufs=1, space="DRAM") as dram,
            tc.tile_pool(name="const", bufs=1) as const,
        ):
            # resident constants
            w1_sb = const.tile([128, FK, H], cdt)
            nc.sync.dma_start(out=w1_sb[:], in_=w1_in[:])
            rw1_sb = const.tile([128, FK, H], cdt)
            nc.sync.dma_start(out=rw1_sb[:], in_=rw1_in[:])
            w2_sb = const.tile([128, HC, OUT], cdt)
            nc.sync.dma_start(out=w2_sb[:], in_=w2_in[:])
            rw2b_sb = const.tile([128, HC + 1, OUT], cdt)
            nc.sync.dma_start(out=rw2b_sb[:], in_=rw2b_in[:])
            bias1_sb = const.tile([128, HC], f32)
            nc.sync.dma_start(out=bias1_sb[:], in_=bias1_in[:])
            idx16_sb = const.tile([128, B * KT * 8], mybir.dt.int16)
            nc.sync.dma_start(out=idx16_sb[:], in_=idx16_in[:])
            idx16b_sb = const.tile([128, B * KT * 8], mybir.dt.int16)
            nc.sync.dma_start(out=idx16b_sb[:], in_=idx16b_in[:])
            ones_sb = const.tile([128, 128], cdt)
            nc.vector.memset(ones_sb[:], 0.0)
            nc.vector.memset(ones_sb[0:1, :], 1.0)
            # dense-2 (h@rw2 + b2) results, computed in phase A
            outd_sb = const.tile([128, B, 128], f32)

            for rep in range(reps):
                hw_locs = [
                    dram.tile([BC * 128, OUT], hwdt, tag=f"hw_loc{g}",
                              name=f"hw_loc{rep}_{g}")
                    for g in range(nchunk)]
                hw_full = dram.tile([NP, OUT], hwdt,
                                    tag="hw_full", name=f"hw_full{rep}")
                # ---------------- phase A: layer 1 + hw + dense2 ------------
                with (
                    tc.tile_pool(name=f"xg_pool{rep}", bufs=2) as xg_pool,
                    tc.tile_pool(name=f"sa_pool{rep}", bufs=3) as sa_pool,
                    tc.tile_pool(name=f"axsb_pool{rep}", bufs=2) as axsb_pool,
                    tc.tile_pool(name=f"hstage_pool{rep}", bufs=3) as hstage_pool,
                    tc.tile_pool(name=f"hwsb_pool{rep}", bufs=3) as hwsb_pool,
                    tc.tile_pool(name=f"ax_psum{rep}", bufs=2,
                                 space="PSUM") as ax_psum,
                    tc.tile_pool(name=f"h_psum{rep}", bufs=2,
                                 space="PSUM") as h_psum,
                    tc.tile_pool(name=f"hw_psum{rep}", bufs=2,
                                 space="PSUM") as hw_psum,
                    tc.tile_pool(name=f"o2_psum{rep}", bufs=2,
                                 space="PSUM") as o2_psum,
                ):
                    for b in range(B):
                        bw = slice(b * 128, (b + 1) * 128)
                        if b % GB == 0:
                            xg = xg_pool.tile([128, GB * KT, F], l1dt,
                                              tag="xg")
                            _gather(xg, x_table, idx16_sb, b,
                                    min(GB, B - b), F)
                        goff = (b % GB) * KT
                        s1_tile = sa_pool.tile([128, KT, 128], l1dt,
                                               tag="s1_sb")
                        nc.sync.dma_start(out=s1_tile[:], in_=s1_in[:, b, :])
                        xt_tile = sa_pool.tile([128, FK, 128], cdt,
                                               tag="xt_tile")
                        nc.sync.dma_start(out=xt_tile[:],
                                          in_=xt_in[:, :, bw])

                        # aggregation in input space: axT[fc] = Xg_chunk.T @ S
                        psum_ax = ax_psum.tile([128, FK, 128], f32,
                                               tag="psum_ax")
                        for fc in range(FK):
                            fs = slice(fc * 128, (fc + 1) * 128)
                            if FP8_L1:
                                for k2 in range(KT // 2):
                                    nc.tensor.matmul(
                                        out=psum_ax[:, fc, :],
                                        lhsT=xg[:, goff + 2 * k2:
                                                goff + 2 * k2 + 2, fs],
                                        rhs=s1_tile[:, 2 * k2:2 * k2 + 2, :],
                                        start=(k2 == 0),
                                        stop=(k2 == KT // 2 - 1),
                                        perf_mode=mybir.MatmulPerfMode.DoubleRow,
                                    )
                            else:
                                for kt in range(KT):
                                    nc.tensor.matmul(
                                        out=psum_ax[:, fc, :],
                                        lhsT=xg[:, goff + kt, fs],
                                        rhs=s1_tile[:, kt, :],
                                        start=(kt == 0),
                                        stop=(kt == KT - 1),
                                    )
                        axT_sb = axsb_pool.tile([128, FK, 128], cdt,
                                                tag="axT_sb")
                        nc.vector.tensor_copy(out=axT_sb[:], in_=psum_ax[:])

                        # dense: hT = relu(W1.T @ axT + RW1.T @ xT + b1c)
                        hT_stage = hstage_pool.tile([128, HC, 128], cdt,
                                                    tag="hT_sb")
                        for half in range(2):
                            psum_h = h_psum.tile([128, HC // 2, 128], f32,
                                                 tag="psum_h")
                            for j in range(HC // 2):
                                hc = half * (HC // 2) + j
                                hs = slice(hc * 128, (hc + 1) * 128)
                                for k in range(FK):
                                    nc.tensor.matmul(
                                        out=psum_h[:, j, :],
                                        lhsT=w1_sb[:, k, hs],
                                        rhs=axT_sb[:, k, :],
                                        start=(k == 0), stop=False)
                                for k in range(FK):
                                    nc.tensor.matmul(
                                        out=psum_h[:, j, :],
                                        lhsT=rw1_sb[:, k, hs],
                                        rhs=xt_tile[:, k, :],
                                        start=False, stop=(k == FK - 1))
                            hw0 = half * (HC // 2)
                            for j in range(HC // 2):
                                hc = hw0 + j
                                nc.scalar.activation(
                                    out=hT_stage[:, hc, :],
                                    in_=psum_h[:, j, :],
                                    func=mybir.ActivationFunctionType.Relu,
                                    bias=bias1_sb[:, hc:hc + 1], scale=1.0)

                        # hw = h @ W2   (node-major), stored fp8 for the halo
                        psum_hw = hw_psum.tile([128, OC, 128], f32,
                                               tag="psum_hw")
                        for oc in range(OC):
                            for hc in range(HC):
                                nc.tensor.matmul(
                                    out=psum_hw[:, oc, :],
                                    lhsT=hT_stage[:, hc, :],
                                    rhs=w2_sb[:, hc, oc * 128:(oc + 1) * 128],
                                    start=(hc == 0), stop=(hc == HC - 1))
                        hw_sb = hwsb_pool.tile([128, OUT], hwdt, tag="hw_sb")
                        nc.vector.tensor_copy(out=hw_sb[:], in_=psum_hw[:])
                        lw = slice((b % BC) * 128, (b % BC + 1) * 128)
                        nc.sync.dma_start(out=hw_locs[b // BC][lw, :],
                                          in_=hw_sb[:])

                        # dense2: outd = h @ rw2 + b2c (phase B adds agg2)
                        psum_o2 = o2_psum.tile([128, OC, 128], f32,
                                               tag="psum_o2")
                        for oc in range(OC):
                            ow = slice(oc * 128, (oc + 1) * 128)
                            for hc in range(HC):
                                nc.tensor.matmul(
                                    out=psum_o2[:, oc, :],
                                    lhsT=hT_stage[:, hc, :],
                                    rhs=rw2b_sb[:, hc, ow],
                                    start=(hc == 0), stop=False)
                            nc.tensor.matmul(
                                out=psum_o2[:, oc, :],
                                lhsT=ones_sb[:],
                                rhs=rw2b_sb[:, HC, ow],
                                start=False, stop=True)
                        nc.vector.tensor_copy(out=outd_sb[:, b, :],
                                              in_=psum_o2[:])

                        # chunked all-gather of hw: issue as soon as the
                        # chunk's blocks are all written
                        if (b + 1) % BC == 0:
                            g = b // BC
                            # chunk-major contiguous region of the table
                            orows = slice(g * C * BC * 128,
                                          (g + 1) * C * BC * 128)
                            if no_collective:
                                nc.gpsimd.dma_start(
                                    out=hw_full[orows, :][0:BC * 128, :],
                                    in_=hw_locs[g][:])
                            else:
                                nc.gpsimd.collective_compute(
                                    "AllGather",
                                    mybir.AluOpType.bypass,
                                    replica_groups=[list(range(C))],
                                    ins=[hw_locs[g][:].opt()],
                                    outs=[hw_full[orows, :].opt()],
                                )

                # ---------------- phase B: layer 2 ----------------
                with (
                    tc.tile_pool(name=f"sb_pool{rep}", bufs=3) as sb_pool,
                    tc.tile_pool(name=f"hwg_pool{rep}", bufs=2) as hwg_pool,
                    tc.tile_pool(name=f"hwb_pool{rep}", bufs=3) as hwb_pool,
                    tc.tile_pool(name=f"osb_pool{rep}", bufs=3) as osb_pool,
                    tc.tile_pool(name=f"o_psum{rep}", bufs=3,
                                 space="PSUM") as o_psum,
                ):
                    for b in range(B):
                        bw = slice(b * 128, (b + 1) * 128)
                        s2_tile = sb_pool.tile([128, KT, 128], cdt,
                                               tag="s2_sb")
                        nc.sync.dma_start(out=s2_tile[:], in_=s2_in[:, b, :])
                        if b % GB == 0:
                            hwg = hwg_pool.tile([128, GB * KT, OUT], hwdt,
                                                tag="hwg")
                            _gather(hwg, hw_full, idx16b_sb, b,
                                    min(GB, B - b), OUT)
                        goff = (b % GB) * KT
                        if FP8_HW:
                            hwb = hwb_pool.tile([128, KT, OUT], cdt,
                                                tag="hwb")
                            nc.vector.tensor_copy(
                                out=hwb[:], in_=hwg[:, goff:goff + KT, :])
                            rhs_t = hwb
                            roff = 0
                        else:
                            rhs_t = hwg
                            roff = goff

                        psum_o = o_psum.tile([128, OC, 128], f32, tag="psum_o")
                        for oc in range(OC):
                            ow = slice(oc * 128, (oc + 1) * 128)
                            for kt in range(KT):
                                nc.tensor.matmul(
                                    out=psum_o[:, oc, :],
                                    lhsT=s2_tile[:, kt, :],
                                    rhs=rhs_t[:, roff + kt,
                                              oc * 128:(oc + 1) * 128],
                                    start=(kt == 0), stop=(kt == KT - 1))
                        out_sb = osb_pool.tile([128, OUT], f32, tag="out_sb")
                        nc.vector.tensor_tensor(
                            out=out_sb[:], in0=psum_o[:, 0, :],
                            in1=outd_sb[:, b, :],
                            op=mybir.AluOpType.add)
                        nc.sync.dma_start(out=out_d[bw, :], in_=out_sb[:])

    nc.compile()
    return nc


# ----------------------------------------------------------------------------
# entry points
# ----------------------------------------------------------------------------
def _run(inputs, trace=False, compute=None, trace_kwargs=None):
    compute = compute or COMPUTE
    cdt, np_cdt = _DT[compute]
    x = np.asarray(inputs["x"])
    cfg = _cfg_from_shapes(x, np.asarray(inputs["w1"]),
                           np.asarray(inputs["w2"]))
    in_maps, KT = _preprocess(
        x, inputs["edge_index"], inputs["edge_weight"],
        inputs["w1"], inputs["b1"], inputs["w2"], inputs["b2"],
        inputs["rw1"], inputs["rb1"], inputs["rw2"], inputs["rb2"],
        cfg, np_cdt)

    key = (tuple(sorted(cfg.items())), KT, compute)
    nc = _prog_cache.get(key)
    if nc is None:
        nc = _build(cfg, KT, cdt)
        _prog_cache[key] = nc

    res = run_bass_kernel_spmd(
        nc, in_maps, core_ids=list(range(C)), trace=trace,
        **(trace_kwargs or {}))

    NL, NLP = cfg["NL"], cfg["NLP"]
    out = np.concatenate(
        [res.results[c]["out"][:NL] for c in range(C)], axis=0)
    return np.ascontiguousarray(out.astype(np.float32)), res


def kernel(**inputs) -> np.ndarray:
    out, _ = _run(inputs, trace=False)
    return out


# revision 23
# speedup vs baseline: 1.9708x; 1.9708x over previous
"""Distributed 2-layer GCN (PyG GCNConv-style) on 8 Trainium2 NeuronCores.

Strategy (matches the sharding hint):
  - Nodes are sharded 2500/core (padded to 2560 = 20 blocks of 128).
  - Edges are partitioned by destination node; per (core, dst-block) the
    incoming edges (+ self loops) are packed into uniform KT tiles of 128
    edge slots. The sym-normalization coefficients are precomputed on the
    host into per-block selection matrices S [128 edge-slots, 128 dst] so
    scatter-add becomes a TensorE matmul.
  - Layer 1 aggregates in INPUT space (A @ x, width F=512) before the W1
    matmul; layer 2 aggregates AFTER the W2 projection (width 128). Both
    minimize gathered bytes. The gathered x and the S coefficients are fp8
    (e4m3) with DoubleRow matmuls: halves gather HBM traffic and doubles
    the aggregation matmul rate (rel err ~1.2e-2 vs 3.2e-3 all-bf16).
  - h@rw2 + b2 is computed in phase A (while h is in SBUF), so phase B
    only needs the gathered hw rows: no h table is kept.
  - The halo exchange (AllGather of the hw = h@W2 table, fp8 = 2.6 MB) is
    CHUNKED: one AllGather per 5 dst-blocks, issued as soon as those hw
    blocks are written, so the collective overlaps the rest of phase A.
  - Weights are replicated; everything is laid out feature-major on the
    host so the kernel needs zero on-device transposes.

kernel(**inputs) takes the FULL unsharded inputs and returns the FULL
[20000, 128] float32 output.
"""

import math

import numpy as np
import ml_dtypes

import concourse.bass as bass
import concourse.mybir as mybir
import concourse.tile as tile
from concourse import bacc
from concourse.bass_utils import run_bass_kernel_spmd

# ----------------------------------------------------------------------------
# configuration
# ----------------------------------------------------------------------------
C = 8          # cores
FP8_L1 = True  # gather x + S1 in fp8 e4m3, DoubleRow aggregation matmuls
FP8_HW = True  # hw table fp8 e4m3 (collective + phase-B gather), upconvert
CHUNKS = 2     # AllGather chunks (must divide B)
GB = 2         # dst-blocks per dma_gather op (GB*KT*128 <= 4096 ring)

_DT = {
    "bf16": (mybir.dt.bfloat16, ml_dtypes.bfloat16),
    "f32": (mybir.dt.float32, np.float32),
    "fp8": (mybir.dt.float8e4, ml_dtypes.float8_e4m3),
}
COMPUTE = "bf16"

_prog_cache: dict = {}


def _cfg_from_shapes(x, w1, w2):
    n, f = x.shape
    h = w1.shape[1]
    out = w2.shape[1]
    assert n % C == 0, n
    nl = n // C                      # real nodes per core
    nlp = ((nl + 127) // 128) * 128  # padded nodes per core
    b = nlp // 128                   # dst blocks per core
    assert f % 128 == 0 and h % 128 == 0 and out % 128 == 0
    return dict(N=n, F=f, H=h, OUT=out, NL=nl, NLP=nlp, B=b, NP=C * nlp,
                FK=f // 128, HC=h // 128, OC=out // 128)


# ----------------------------------------------------------------------------
# host-side preprocessing: graph partition + norm coefficients + layouts
# ----------------------------------------------------------------------------
def _preprocess(x, edge_index, edge_weight, w1, b1, w2, b2, rw1, rb1, rw2, rb2,
                cfg, np_cdt):
    N, F, H, OUT = cfg["N"], cfg["F"], cfg["H"], cfg["OUT"]
    NL, NLP, B, NP = cfg["NL"], cfg["NLP"], cfg["B"], cfg["NP"]
    HC, FK = cfg["HC"], cfg["FK"]
    np_l1 = ml_dtypes.float8_e4m3 if FP8_L1 else np_cdt

    row = np.asarray(edge_index[0], dtype=np.int64)
    col = np.asarray(edge_index[1], dtype=np.int64)
    ew = np.asarray(edge_weight, dtype=np.float32)

    # symmetric normalization, exactly like the reference (self loop weight 1)
    deg = np.bincount(col, weights=ew.astype(np.float64), minlength=N) + 1.0
    deg = deg.astype(np.float32)
    dis = np.where(deg > 0, 1.0 / np.sqrt(np.where(deg > 0, deg, 1.0)), 0.0)
    dis = dis.astype(np.float32)

    loop = np.arange(N, dtype=np.int64)
    srcs = np.concatenate([row, loop])
    dsts = np.concatenate([col, loop])
    norms = np.concatenate([dis[row] * ew * dis[col], dis * dis])

    # padded node ids: node g lives on core g//NL at local slot g%NL
    src_pad = (srcs // NL) * NLP + (srcs % NL)
    core = dsts // NL
    local = dsts % NL
    blk = local // 128
    dloc = local % 128

    key = (core * B + blk).astype(np.int64)
    order = np.argsort(key, kind="stable")
    key_s = key[order]
    counts = np.bincount(key_s, minlength=C * B)
    starts = np.zeros(C * B, dtype=np.int64)
    np.cumsum(counts[:-1], out=starts[1:])
    pos = np.arange(key_s.size, dtype=np.int64) - starts[key_s]

    KT = max(1, int(math.ceil(counts.max() / 128)))
    if KT % 2 and FP8_L1:
        KT += 1  # even KT for DoubleRow pairs

    src_s = src_pad[order].astype(np.int32)
    core_s = core[order]
    blk_s = blk[order]
    dloc_s = dloc[order]
    norm_s = norms[order]
    kt_s = pos // 128
    p_s = pos % 128

    # int16 indices for dma_gather: slot i of block b -> [i%16, b*KT*8 + i//16],
    # replicated across the 8 groups of 16 partitions
    assert NP < 2 ** 15
    slot = kt_s * 128 + p_s
    idx16 = np.zeros((C, 16, B * KT * 8), dtype=np.int16)
    idx16[core_s, slot % 16, blk_s * (KT * 8) + slot // 16] = \
        src_s.astype(np.int16)
    idx16_all = np.tile(idx16, (1, 8, 1))

    # phase-B indices: hw table is CHUNK-major (each chunked AllGather
    # writes a contiguous [C*BC*128, OUT] region):
    #   row(src) = g*C*BC*128 + core*BC*128 + (lp % (BC*128)), g = lp//(BC*128)
    nchunk = CHUNKS if B % CHUNKS == 0 else 1
    BCB = (B // nchunk) * 128
    src_core = srcs // NL
    src_lp = srcs % NL
    row_b = ((src_lp // BCB) * C * BCB + src_core * BCB + src_lp % BCB)
    row_b_s = row_b[order].astype(np.int16)
    idx16b = np.zeros((C, 16, B * KT * 8), dtype=np.int16)
    idx16b[core_s, slot % 16, blk_s * (KT * 8) + slot // 16] = row_b_s
    idx16b_all = np.tile(idx16b, (1, 8, 1))

    # S coefficients, partition(slot)-major: S[c, p, b, kt, d]
    S_all = np.zeros((C, 128, B, KT, 128), dtype=np.float32)
    S_all[core_s, p_s, blk_s, kt_s, dloc_s] = norm_s
    s1_all = S_all.astype(np_l1)
    s2_all = S_all.astype(np_cdt)

    # padded, replicated x table [NP, F] (fp8 when FP8_L1)
    x = np.asarray(x, dtype=np.float32)
    x_table = np.zeros((NP, F), dtype=np.float32)
    x_table.reshape(C, NLP, F)[:, :NL] = x.reshape(C, NL, F)
    x_table = x_table.astype(np_l1)

    # feature-major x per core (bf16): xT[p, k, n] = x_core[n, k*128+p]
    xT_all = np.ascontiguousarray(
        np.zeros((C, NLP, F), np.float32).reshape(C, NLP, FK, 128)
        .transpose(0, 3, 2, 1)).astype(np_cdt)
    xt_f32 = np.zeros((C, NLP, F), dtype=np.float32)
    xt_f32[:, :NL] = x.reshape(C, NL, F)
    xT_all = np.ascontiguousarray(
        xt_f32.reshape(C, NLP, FK, 128).transpose(0, 3, 2, 1)).astype(np_cdt)

    w1 = np.asarray(w1, np.float32)
    rw1 = np.asarray(rw1, np.float32)
    w2 = np.asarray(w2, np.float32)
    rw2 = np.asarray(rw2, np.float32)
    b1c = (np.asarray(b1, np.float32) + np.asarray(rb1, np.float32))
    b2c = (np.asarray(b2, np.float32) + np.asarray(rb2, np.float32))

    # [128, FK, H] : w1_in[p, k, j] = w1[k*128+p, j]
    w1_in = np.ascontiguousarray(
        w1.reshape(FK, 128, H).transpose(1, 0, 2)).astype(np_cdt)
    rw1_in = np.ascontiguousarray(
        rw1.reshape(FK, 128, H).transpose(1, 0, 2)).astype(np_cdt)
    w2_in = np.ascontiguousarray(
        w2.reshape(HC, 128, OUT).transpose(1, 0, 2)).astype(np_cdt)
    rw2b_in = np.zeros((128, HC + 1, OUT), dtype=np.float32)
    rw2b_in[:, :HC] = rw2.reshape(HC, 128, OUT).transpose(1, 0, 2)
    rw2b_in[0, HC, :] = b2c
    rw2b_in = rw2b_in.astype(np_cdt)

    bias1_in = np.ascontiguousarray(b1c.reshape(HC, 128).T).astype(np.float32)

    in_maps = []
    for c in range(C):
        in_maps.append({
            "x_table": x_table,
            "idx16_in": np.ascontiguousarray(idx16_all[c]),
            "idx16b_in": np.ascontiguousarray(idx16b_all[c]),
            "s1_in": np.ascontiguousarray(
                s1_all[c].reshape(128, B, KT * 128)),
            "s2_in": np.ascontiguousarray(
                s2_all[c].reshape(128, B, KT * 128)),
            "xt_in": np.ascontiguousarray(xT_all[c]),
            "w1_in": w1_in,
            "rw1_in": rw1_in,
            "w2_in": w2_in,
            "rw2b_in": rw2b_in,
            "bias1_in": bias1_in,
        })
    return in_maps, KT


# ----------------------------------------------------------------------------
# device program
# ----------------------------------------------------------------------------
def _build(cfg, KT, cdt, debug_out=False, reps=1, no_collective=False):
    F, H, OUT = cfg["F"], cfg["H"], cfg["OUT"]
    NLP, B, NP = cfg["NLP"], cfg["B"], cfg["NP"]
    FK, HC, OC = cfg["FK"], cfg["HC"], cfg["OC"]
    f32 = mybir.dt.float32
    l1dt = mybir.dt.float8e4 if FP8_L1 else cdt
    hwdt = mybir.dt.float8e4 if FP8_HW else cdt
    nchunk = CHUNKS if B % CHUNKS == 0 else 1
    BC = B // nchunk          # blocks per collective chunk
    assert GB * KT * 128 <= 4096

    nc = bacc.Bacc("TRN2", target_bir_lowering=False, debug=False,
                   enable_asserts=False, num_devices=C,
                   dynamic_dma_scratch_size=65536, num_swdge_queues=2)

    x_table = nc.dram_tensor("x_table", [NP, F], l1dt, kind="ExternalInput")
    idx16_in = nc.dram_tensor("idx16_in", [128, B * KT * 8], mybir.dt.int16,
                              kind="ExternalInput")
    idx16b_in = nc.dram_tensor("idx16b_in", [128, B * KT * 8], mybir.dt.int16,
                               kind="ExternalInput")
    s1_in = nc.dram_tensor("s1_in", [128, B, KT * 128], l1dt,
                           kind="ExternalInput")
    s2_in = nc.dram_tensor("s2_in", [128, B, KT * 128], cdt,
                           kind="ExternalInput")
    xt_in = nc.dram_tensor("xt_in", [128, FK, NLP], cdt, kind="ExternalInput")
    w1_in = nc.dram_tensor("w1_in", [128, FK, H], cdt, kind="ExternalInput")
    rw1_in = nc.dram_tensor("rw1_in", [128, FK, H], cdt, kind="ExternalInput")
    w2_in = nc.dram_tensor("w2_in", [128, HC, OUT], cdt, kind="ExternalInput")
    rw2b_in = nc.dram_tensor("rw2b_in", [128, HC + 1, OUT], cdt,
                             kind="ExternalInput")
    bias1_in = nc.dram_tensor("bias1_in", [128, HC], f32,
                              kind="ExternalInput")
    out_d = nc.dram_tensor("out", [NLP, OUT], f32, kind="ExternalOutput")

    def _gather(out_tile, table, idx16_sb, b0, nblk, elem, ops=2,
                queue=None):
        # `ops` desc-gen instructions per block; smaller ops pipeline deeper
        # in the 4096-desc SWDGE ring
        for blk in range(nblk):
            b = b0 + blk
            step = KT // ops
            for o in range(ops):
                lo, hi = o * step, (o + 1) * step if o < ops - 1 else KT
                nc.gpsimd.dma_gather(
                    out_ap=out_tile[:, blk * KT + lo:blk * KT + hi],
                    in_ap=table[:],
                    idxs_ap=idx16_sb[:, b * KT * 8 + lo * 8:
                                     b * KT * 8 + hi * 8],
                    num_idxs=(hi - lo) * 128, num_idxs_reg=(hi - lo) * 128,
                    elem_size=elem, single_packet=False,
                    queue_num=b % 2 if queue is None else queue)

    with tile.TileContext(nc) as tc:
        with (
            tc.tile_pool(name="dram", bufs=1, space="DRAM") as dram,
            tc.tile_pool(name="const", bufs=1) as const,
        ):
            # resident constants
            w1_sb = const.tile([128, FK, H], cdt)
            nc.sync.dma_start(out=w1_sb[:], in_=w1_in[:])
            rw1_sb = const.tile([128, FK, H], cdt)
            nc.sync.dma_start(out=rw1_sb[:], in_=rw1_in[:])
            w2_sb = const.tile([128, HC, OUT], cdt)
            nc.sync.dma_start(out=w2_sb[:], in_=w2_in[:])
            rw2b_sb = const.tile([128, HC + 1, OUT], cdt)
            nc.sync.dma_start(out=rw2b_sb[:], in_=rw2b_in[:])
            bias1_sb = const.tile([128, HC], f32)
            nc.sync.dma_start(out=bias1_sb[:], in_=bias1_in[:])
            idx16_sb = const.tile([128, B * KT * 8], mybir.dt.int16)
            nc.sync.dma_start(out=idx16_sb[:], in_=idx16_in[:])
            idx16b_sb = const.tile([128, B * KT * 8], mybir.dt.int16)
            nc.sync.dma_start(out=idx16b_sb[:], in_=idx16b_in[:])
            ones_sb = const.tile([128, 128], cdt)
            nc.vector.memset(ones_sb[:], 0.0)
            nc.vector.memset(ones_sb[0:1, :], 1.0)
            # dense-2 (h@rw2 + b2) results, computed in phase A
            outd_sb = const.tile([128, B, 128], f32)

            for rep in range(reps):
                hw_locs = [
                    dram.tile([BC * 128, OUT], hwdt, tag=f"hw_loc{g}",
                              name=f"hw_loc{rep}_{g}")
                    for g in range(nchunk)]
                hw_full = dram.tile([NP, OUT], hwdt,
                                    tag="hw_full", name=f"hw_full{rep}")
                if FP8_HW:
                    hw_pad = dram.tile([NP, 2 * OUT], hwdt, tag="hw_pad",
                                       name=f"hw_pad{rep}")
                # ---------------- phase A: layer 1 + hw + dense2 ------------
                with (
                    tc.tile_pool(name=f"xg_pool{rep}", bufs=7) as xg_pool,
                    tc.tile_pool(name=f"sa_pool{rep}", bufs=3) as sa_pool,
                    tc.tile_pool(name=f"axsb_pool{rep}", bufs=2) as axsb_pool,
                    tc.tile_pool(name=f"hstage_pool{rep}", bufs=3) as hstage_pool,
                    tc.tile_pool(name=f"hwsb_pool{rep}", bufs=3) as hwsb_pool,
                    tc.tile_pool(name=f"ax_psum{rep}", bufs=2,
                                 space="PSUM") as ax_psum,
                    tc.tile_pool(name=f"h_psum{rep}", bufs=2,
                                 space="PSUM") as h_psum,
                    tc.tile_pool(name=f"hw_psum{rep}", bufs=2,
                                 space="PSUM") as hw_psum,
                    tc.tile_pool(name=f"o2_psum{rep}", bufs=2,
                                 space="PSUM") as o2_psum,
                ):
                    for b in range(B):
                        bw = slice(b * 128, (b + 1) * 128)
                        if b % GB == 0:
                            xg = xg_pool.tile([128, GB * KT, F], l1dt,
                                              tag="xg")
                            _gather(xg, x_table, idx16_sb, b,
                                    min(GB, B - b), F)
                        goff = (b % GB) * KT
                        s1_tile = sa_pool.tile([128, KT, 128], l1dt,
                                               tag="s1_sb")
                        nc.scalar.dma_start(out=s1_tile[:],
                                            in_=s1_in[:, b, :])
                        xt_tile = sa_pool.tile([128, FK, 128], cdt,
                                               tag="xt_tile")
                        nc.scalar.dma_start(out=xt_tile[:],
                                            in_=xt_in[:, :, bw])

                        # aggregation in input space: axT[fc] = Xg_chunk.T @ S
                        psum_ax = ax_psum.tile([128, FK, 128], f32,
                                               tag="psum_ax")
                        for fc in range(FK):
                            fs = slice(fc * 128, (fc + 1) * 128)
                            if FP8_L1:
                                for k2 in range(KT // 2):
                                    nc.tensor.matmul(
                                        out=psum_ax[:, fc, :],
                                        lhsT=xg[:, goff + 2 * k2:
                                                goff + 2 * k2 + 2, fs],
                                        rhs=s1_tile[:, 2 * k2:2 * k2 + 2, :],
                                        start=(k2 == 0),
                                        stop=(k2 == KT // 2 - 1),
                                        perf_mode=mybir.MatmulPerfMode.DoubleRow,
                                    )
                            else:
                                for kt in range(KT):
                                    nc.tensor.matmul(
                                        out=psum_ax[:, fc, :],
                                        lhsT=xg[:, goff + kt, fs],
                                        rhs=s1_tile[:, kt, :],
                                        start=(kt == 0),
                                        stop=(kt == KT - 1),
                                    )
                        axT_sb = axsb_pool.tile([128, FK, 128], cdt,
                                                tag="axT_sb")
                        nc.vector.tensor_copy(out=axT_sb[:], in_=psum_ax[:])

                        # dense: hT = relu(W1.T @ axT + RW1.T @ xT + b1c)
                        hT_stage = hstage_pool.tile([128, HC, 128], cdt,
                                                    tag="hT_sb")
                        for half in range(2):
                            psum_h = h_psum.tile([128, HC // 2, 128], f32,
                                                 tag="psum_h")
                            for j in range(HC // 2):
                                hc = half * (HC // 2) + j
                                hs = slice(hc * 128, (hc + 1) * 128)
                                for k in range(FK):
                                    nc.tensor.matmul(
                                        out=psum_h[:, j, :],
                                        lhsT=w1_sb[:, k, hs],
                                        rhs=axT_sb[:, k, :],
                                        start=(k == 0), stop=False)
                                for k in range(FK):
                                    nc.tensor.matmul(
                                        out=psum_h[:, j, :],
                                        lhsT=rw1_sb[:, k, hs],
                                        rhs=xt_tile[:, k, :],
                                        start=False, stop=(k == FK - 1))
                            hw0 = half * (HC // 2)
                            for j in range(HC // 2):
                                hc = hw0 + j
                                nc.scalar.activation(
                                    out=hT_stage[:, hc, :],
                                    in_=psum_h[:, j, :],
                                    func=mybir.ActivationFunctionType.Relu,
                                    bias=bias1_sb[:, hc:hc + 1], scale=1.0)

                        # hw = h @ W2   (node-major), stored fp8 for the halo
                        psum_hw = hw_psum.tile([128, OC, 128], f32,
                                               tag="psum_hw")
                        for oc in range(OC):
                            for hc in range(HC):
                                nc.tensor.matmul(
                                    out=psum_hw[:, oc, :],
                                    lhsT=hT_stage[:, hc, :],
                                    rhs=w2_sb[:, hc, oc * 128:(oc + 1) * 128],
                                    start=(hc == 0), stop=(hc == HC - 1))
                        hw_sb = hwsb_pool.tile([128, OUT], hwdt, tag="hw_sb")
                        nc.vector.tensor_copy(out=hw_sb[:], in_=psum_hw[:])
                        lw = slice((b % BC) * 128, (b % BC + 1) * 128)
                        nc.scalar.dma_start(out=hw_locs[b // BC][lw, :],
                                            in_=hw_sb[:])

                        # dense2: outd = h @ rw2 + b2c (phase B adds agg2)
                        psum_o2 = o2_psum.tile([128, OC, 128], f32,
                                               tag="psum_o2")
                        for oc in range(OC):
                            ow = slice(oc * 128, (oc + 1) * 128)
                            for hc in range(HC):
                                nc.tensor.matmul(
                                    out=psum_o2[:, oc, :],
                                    lhsT=hT_stage[:, hc, :],
                                    rhs=rw2b_sb[:, hc, ow],
                                    start=(hc == 0), stop=False)
                            nc.tensor.matmul(
                                out=psum_o2[:, oc, :],
                                lhsT=ones_sb[:],
                                rhs=rw2b_sb[:, HC, ow],
                                start=False, stop=True)
                        nc.vector.tensor_copy(out=outd_sb[:, b, :],
                                              in_=psum_o2[:])

                # chunked all-gather of hw on the SP stream: each chunk
                # launches as soon as its blocks' hw rows are written, and
                # the SP stream has no other phase-A work to block.
                sp_eng = nc.engines[mybir.EngineType.SP]
                for g in range(nchunk):
                    orows = slice(g * C * BC * 128, (g + 1) * C * BC * 128)
                    if no_collective:
                        nc.scalar.dma_start(
                            out=hw_full[orows, :][0:BC * 128, :],
                            in_=hw_locs[g][:])
                    else:
                        nc.gpsimd.collective_compute(
                            "AllGather",
                            mybir.AluOpType.bypass,
                            replica_groups=[list(range(C))],
                            ins=[hw_locs[g][:].opt()],
                            outs=[hw_full[orows, :].opt()],
                        )
                    if FP8_HW:
                        # expand compact fp8 rows into 256B-padded rows so
                        # the phase-B gather meets the 256B stride rule
                        # (both halves written; the pad half is unused but
                        # keeps the finite-checker happy)
                        sp_eng.dma_start(
                            out=hw_pad[orows, 0:OUT],
                            in_=hw_full[orows, :])
                        sp_eng.dma_start(
                            out=hw_pad[orows, OUT:2 * OUT],
                            in_=hw_full[orows, :])

                # ---------------- phase B: layer 2 ----------------
                with (
                    tc.tile_pool(name=f"sb_pool{rep}", bufs=3) as sb_pool,
                    tc.tile_pool(name=f"hwg_pool{rep}", bufs=2) as hwg_pool,
                    tc.tile_pool(name=f"hwb_pool{rep}", bufs=3) as hwb_pool,
                    tc.tile_pool(name=f"osb_pool{rep}", bufs=3) as osb_pool,
                    tc.tile_pool(name=f"o_psum{rep}", bufs=3,
                                 space="PSUM") as o_psum,
                ):
                    for b in range(B):
                        bw = slice(b * 128, (b + 1) * 128)
                        s2_tile = sb_pool.tile([128, KT, 128], cdt,
                                               tag="s2_sb")
                        nc.sync.dma_start(out=s2_tile[:], in_=s2_in[:, b, :])
                        if b % GB == 0:
                            gw = 2 * OUT if FP8_HW else OUT
                            hwg = hwg_pool.tile([128, GB * KT, gw], hwdt,
                                                tag="hwg")
                            tbl = hw_pad if FP8_HW else hw_full
                            _gather(hwg, tbl, idx16b_sb, b,
                                    min(GB, B - b), gw, ops=1,
                                    queue=(b // GB) % 2)
                        goff = (b % GB) * KT
                        if FP8_HW:
                            # payload = [row v | row v+1]; keep first half,
                            # upconvert fp8 -> bf16 for the matmul
                            hwb = hwb_pool.tile([128, KT, OUT], cdt,
                                                tag="hwb")
                            nc.scalar.activation(
                                out=hwb[:],
                                in_=hwg[:, goff:goff + KT, 0:OUT],
                                func=mybir.ActivationFunctionType.Copy,
                                scale=1.0)
                            rhs_t = hwb
                            roff = 0
                        else:
                            rhs_t = hwg
                            roff = goff

                        psum_o = o_psum.tile([128, OC, 128], f32, tag="psum_o")
                        for oc in range(OC):
                            ow = slice(oc * 128, (oc + 1) * 128)
                            for kt in range(KT):
                                nc.tensor.matmul(
                                    out=psum_o[:, oc, :],
                                    lhsT=s2_tile[:, kt, :],
                                    rhs=rhs_t[:, roff + kt,
                                              oc * 128:(oc + 1) * 128],
                                    start=(kt == 0), stop=(kt == KT - 1))
                        out_sb = osb_pool.tile([128, OUT], f32, tag="out_sb")
                        nc.vector.tensor_tensor(
                            out=out_sb[:], in0=psum_o[:, 0, :],
                            in1=outd_sb[:, b, :],
                            op=mybir.AluOpType.add)
                        nc.sync.dma_start(out=out_d[bw, :], in_=out_sb[:])

    nc.compile()
    return nc


# ----------------------------------------------------------------------------
# entry points
# ----------------------------------------------------------------------------
def _run(inputs, trace=False, compute=None, trace_kwargs=None):
    compute = compute or COMPUTE
    cdt, np_cdt = _DT[compute]
    x = np.asarray(inputs["x"])
    cfg = _cfg_from_shapes(x, np.asarray(inputs["w1"]),
                           np.asarray(inputs["w2"]))
    in_maps, KT = _preprocess(
        x, inputs["edge_index"], inputs["edge_weight"],
        inputs["w1"], inputs["b1"], inputs["w2"], inputs["b2"],
        inputs["rw1"], inputs["rb1"], inputs["rw2"], inputs["rb2"],
        cfg, np_cdt)

    key = (tuple(sorted(cfg.items())), KT, compute)
    nc = _prog_cache.get(key)
    if nc is None:
        nc = _build(cfg, KT, cdt)
        _prog_cache[key] = nc

    res = run_bass_kernel_spmd(
        nc, in_maps, core_ids=list(range(C)), trace=trace,
        **(trace_kwargs or {}))

    NL, NLP = cfg["NL"], cfg["NLP"]
    out = np.concatenate(
        [res.results[c]["out"][:NL] for c in range(C)], axis=0)
    return np.ascontiguousarray(out.astype(np.float32)), res


def kernel(**inputs) -> np.ndarray:
    out, _ = _run(inputs, trace=False)
    return out


# revision 24
# speedup vs baseline: 2.8707x; 1.4566x over previous
"""Distributed 2-layer GCN (PyG GCNConv-style) on 8 Trainium2 NeuronCores.

Strategy (matches the sharding hint):
  - Nodes are sharded 2500/core (padded to 2560 = 20 blocks of 128).
  - Edges are partitioned by destination node; per (core, dst-block) the
    incoming edges (+ self loops) are packed into uniform KT tiles of 128
    edge slots. The sym-normalization coefficients are precomputed on the
    host into per-block selection matrices S [128 edge-slots, 128 dst] so
    scatter-add becomes a TensorE matmul.
  - Layer 1 aggregates in INPUT space (A @ x, width F=512) before the W1
    matmul; layer 2 aggregates AFTER the W2 projection (width 128). Both
    minimize gathered bytes. The gathered x and the S coefficients are fp8
    (e4m3) with DoubleRow matmuls: halves gather HBM traffic and doubles
    the aggregation matmul rate (rel err ~1.2e-2 vs 3.2e-3 all-bf16).
  - h@rw2 + b2 is computed in phase A (while h is in SBUF), so phase B
    only needs the gathered hw rows: no h table is kept.
  - The halo exchange (AllGather of the hw = h@W2 table, fp8 = 2.6 MB) is
    CHUNKED: one AllGather per 5 dst-blocks, issued as soon as those hw
    blocks are written, so the collective overlaps the rest of phase A.
  - Weights are replicated; everything is laid out feature-major on the
    host so the kernel needs zero on-device transposes.

kernel(**inputs) takes the FULL unsharded inputs and returns the FULL
[20000, 128] float32 output.
"""

import math

import numpy as np
import ml_dtypes

import concourse.bass as bass
import concourse.mybir as mybir
import concourse.tile as tile
from concourse import bacc
from concourse.bass_utils import run_bass_kernel_spmd

# ----------------------------------------------------------------------------
# configuration
# ----------------------------------------------------------------------------
C = 8          # cores
FP8_L1 = True  # gather x + S1 in fp8 e4m3, DoubleRow aggregation matmuls
FP8_HW = True  # hw table fp8 e4m3 (collective + phase-B gather), upconvert
CHUNKS = 2     # AllGather chunks (must divide B)
GB = 2         # dst-blocks per xg tile
OPS_A = 2      # dma_gather desc-gen ops per block in phase A

_DT = {
    "bf16": (mybir.dt.bfloat16, ml_dtypes.bfloat16),
    "f32": (mybir.dt.float32, np.float32),
    "fp8": (mybir.dt.float8e4, ml_dtypes.float8_e4m3),
}
COMPUTE = "bf16"

_prog_cache: dict = {}


def _cfg_from_shapes(x, w1, w2):
    n, f = x.shape
    h = w1.shape[1]
    out = w2.shape[1]
    assert n % C == 0, n
    nl = n // C                      # real nodes per core
    nlp = ((nl + 127) // 128) * 128  # padded nodes per core
    b = nlp // 128                   # dst blocks per core
    assert f % 128 == 0 and h % 128 == 0 and out % 128 == 0
    return dict(N=n, F=f, H=h, OUT=out, NL=nl, NLP=nlp, B=b, NP=C * nlp,
                FK=f // 128, HC=h // 128, OC=out // 128)


# ----------------------------------------------------------------------------
# host-side preprocessing: graph partition + norm coefficients + layouts
# ----------------------------------------------------------------------------
def _preprocess(x, edge_index, edge_weight, w1, b1, w2, b2, rw1, rb1, rw2, rb2,
                cfg, np_cdt):
    N, F, H, OUT = cfg["N"], cfg["F"], cfg["H"], cfg["OUT"]
    NL, NLP, B, NP = cfg["NL"], cfg["NLP"], cfg["B"], cfg["NP"]
    HC, FK = cfg["HC"], cfg["FK"]
    np_l1 = ml_dtypes.float8_e4m3 if FP8_L1 else np_cdt

    row = np.asarray(edge_index[0], dtype=np.int64)
    col = np.asarray(edge_index[1], dtype=np.int64)
    ew = np.asarray(edge_weight, dtype=np.float32)

    # symmetric normalization, exactly like the reference (self loop weight 1)
    deg = np.bincount(col, weights=ew.astype(np.float64), minlength=N) + 1.0
    deg = deg.astype(np.float32)
    dis = np.where(deg > 0, 1.0 / np.sqrt(np.where(deg > 0, deg, 1.0)), 0.0)
    dis = dis.astype(np.float32)

    loop = np.arange(N, dtype=np.int64)
    srcs = np.concatenate([row, loop])
    dsts = np.concatenate([col, loop])
    norms = np.concatenate([dis[row] * ew * dis[col], dis * dis])

    # padded node ids: node g lives on core g//NL at local slot g%NL
    src_pad = (srcs // NL) * NLP + (srcs % NL)
    core = dsts // NL
    local = dsts % NL
    blk = local // 128
    dloc = local % 128

    key = (core * B + blk).astype(np.int64)
    order = np.argsort(key, kind="stable")
    key_s = key[order]
    counts = np.bincount(key_s, minlength=C * B)
    starts = np.zeros(C * B, dtype=np.int64)
    np.cumsum(counts[:-1], out=starts[1:])
    pos = np.arange(key_s.size, dtype=np.int64) - starts[key_s]

    KT = max(1, int(math.ceil(counts.max() / 128)))
    if KT % 2 and FP8_L1:
        KT += 1  # even KT for DoubleRow pairs

    src_s = src_pad[order].astype(np.int32)
    core_s = core[order]
    blk_s = blk[order]
    dloc_s = dloc[order]
    norm_s = norms[order]
    kt_s = pos // 128
    p_s = pos % 128

    # int16 indices for dma_gather: slot i of block b -> [i%16, b*KT*8 + i//16],
    # replicated across the 8 groups of 16 partitions
    assert NP < 2 ** 15
    slot = kt_s * 128 + p_s
    idx16 = np.zeros((C, 16, B * KT * 8), dtype=np.int16)
    idx16[core_s, slot % 16, blk_s * (KT * 8) + slot // 16] = \
        src_s.astype(np.int16)
    idx16_all = np.tile(idx16, (1, 8, 1))

    # phase-B indices: hw table is CHUNK-major (each chunked AllGather
    # writes a contiguous [C*BC*128, OUT] region):
    #   row(src) = g*C*BC*128 + core*BC*128 + (lp % (BC*128)), g = lp//(BC*128)
    nchunk = CHUNKS if B % CHUNKS == 0 else 1
    BCB = (B // nchunk) * 128
    src_core = srcs // NL
    src_lp = srcs % NL
    row_b = ((src_lp // BCB) * C * BCB + src_core * BCB + src_lp % BCB)
    row_b_s = row_b[order].astype(np.int16)
    idx16b = np.zeros((C, 16, B * KT * 8), dtype=np.int16)
    idx16b[core_s, slot % 16, blk_s * (KT * 8) + slot // 16] = row_b_s
    idx16b_all = np.tile(idx16b, (1, 8, 1))

    # S coefficients, partition(slot)-major: S[c, p, b, kt, d]
    S_all = np.zeros((C, 128, B, KT, 128), dtype=np.float32)
    S_all[core_s, p_s, blk_s, kt_s, dloc_s] = norm_s
    s1_all = S_all.astype(np_l1)
    s2_all = S_all.astype(np_cdt)

    # padded, replicated x table [NP, F] (fp8 when FP8_L1)
    x = np.asarray(x, dtype=np.float32)
    x_table = np.zeros((NP, F), dtype=np.float32)
    x_table.reshape(C, NLP, F)[:, :NL] = x.reshape(C, NL, F)
    x_table = x_table.astype(np_l1)

    # feature-major x per core (bf16): xT[p, k, n] = x_core[n, k*128+p]
    xT_all = np.ascontiguousarray(
        np.zeros((C, NLP, F), np.float32).reshape(C, NLP, FK, 128)
        .transpose(0, 3, 2, 1)).astype(np_cdt)
    xt_f32 = np.zeros((C, NLP, F), dtype=np.float32)
    xt_f32[:, :NL] = x.reshape(C, NL, F)
    xT_all = np.ascontiguousarray(
        xt_f32.reshape(C, NLP, FK, 128).transpose(0, 3, 2, 1)).astype(np_cdt)

    w1 = np.asarray(w1, np.float32)
    rw1 = np.asarray(rw1, np.float32)
    w2 = np.asarray(w2, np.float32)
    rw2 = np.asarray(rw2, np.float32)
    b1c = (np.asarray(b1, np.float32) + np.asarray(rb1, np.float32))
    b2c = (np.asarray(b2, np.float32) + np.asarray(rb2, np.float32))

    # [128, FK, H] : w1_in[p, k, j] = w1[k*128+p, j]
    w1_in = np.ascontiguousarray(
        w1.reshape(FK, 128, H).transpose(1, 0, 2)).astype(np_cdt)
    rw1_in = np.ascontiguousarray(
        rw1.reshape(FK, 128, H).transpose(1, 0, 2)).astype(np_cdt)
    w2_in = np.ascontiguousarray(
        w2.reshape(HC, 128, OUT).transpose(1, 0, 2)).astype(np_cdt)
    rw2b_in = np.zeros((128, HC + 1, OUT), dtype=np.float32)
    rw2b_in[:, :HC] = rw2.reshape(HC, 128, OUT).transpose(1, 0, 2)
    rw2b_in[0, HC, :] = b2c
    rw2b_in = rw2b_in.astype(np_cdt)

    bias1_in = np.ascontiguousarray(b1c.reshape(HC, 128).T).astype(np.float32)

    in_maps = []
    for c in range(C):
        in_maps.append({
            "x_table": x_table,
            "idx16_in": np.ascontiguousarray(idx16_all[c]),
            "idx16b_in": np.ascontiguousarray(idx16b_all[c]),
            "s1_in": np.ascontiguousarray(
                s1_all[c].reshape(128, B, KT * 128)),
            "s2_in": np.ascontiguousarray(
                s2_all[c].reshape(128, B, KT * 128)),
            "xt_in": np.ascontiguousarray(xT_all[c]),
            "w1_in": w1_in,
            "rw1_in": rw1_in,
            "w2_in": w2_in,
            "rw2b_in": rw2b_in,
            "bias1_in": bias1_in,
        })
    return in_maps, KT


# ----------------------------------------------------------------------------
# device program
# ----------------------------------------------------------------------------
def _build(cfg, KT, cdt, debug_out=False, reps=1, no_collective=False):
    F, H, OUT = cfg["F"], cfg["H"], cfg["OUT"]
    NLP, B, NP = cfg["NLP"], cfg["B"], cfg["NP"]
    FK, HC, OC = cfg["FK"], cfg["HC"], cfg["OC"]
    f32 = mybir.dt.float32
    l1dt = mybir.dt.float8e4 if FP8_L1 else cdt
    hwdt = mybir.dt.float8e4 if FP8_HW else cdt
    nchunk = CHUNKS if B % CHUNKS == 0 else 1
    BC = B // nchunk          # blocks per collective chunk
    assert GB * KT * 128 <= 4096

    nc = bacc.Bacc("TRN2", target_bir_lowering=False, debug=False,
                   enable_asserts=False, num_devices=C,
                   dynamic_dma_scratch_size=65536, num_swdge_queues=2)

    x_table = nc.dram_tensor("x_table", [NP, F], l1dt, kind="ExternalInput")
    idx16_in = nc.dram_tensor("idx16_in", [128, B * KT * 8], mybir.dt.int16,
                              kind="ExternalInput")
    idx16b_in = nc.dram_tensor("idx16b_in", [128, B * KT * 8], mybir.dt.int16,
                               kind="ExternalInput")
    s1_in = nc.dram_tensor("s1_in", [128, B, KT * 128], l1dt,
                           kind="ExternalInput")
    s2_in = nc.dram_tensor("s2_in", [128, B, KT * 128], cdt,
                           kind="ExternalInput")
    xt_in = nc.dram_tensor("xt_in", [128, FK, NLP], cdt, kind="ExternalInput")
    w1_in = nc.dram_tensor("w1_in", [128, FK, H], cdt, kind="ExternalInput")
    rw1_in = nc.dram_tensor("rw1_in", [128, FK, H], cdt, kind="ExternalInput")
    w2_in = nc.dram_tensor("w2_in", [128, HC, OUT], cdt, kind="ExternalInput")
    rw2b_in = nc.dram_tensor("rw2b_in", [128, HC + 1, OUT], cdt,
                             kind="ExternalInput")
    bias1_in = nc.dram_tensor("bias1_in", [128, HC], f32,
                              kind="ExternalInput")
    out_d = nc.dram_tensor("out", [NLP, OUT], f32, kind="ExternalOutput")

    def _gather(out_tile, table, idx16_sb, b0, nblk, elem, ops=2,
                queue=None):
        # `ops` desc-gen instructions per block; smaller ops pipeline deeper
        # in the 4096-desc SWDGE ring
        for blk in range(nblk):
            b = b0 + blk
            step = KT // ops
            for o in range(ops):
                lo, hi = o * step, (o + 1) * step if o < ops - 1 else KT
                nc.gpsimd.dma_gather(
                    out_ap=out_tile[:, blk * KT + lo:blk * KT + hi],
                    in_ap=table[:],
                    idxs_ap=idx16_sb[:, b * KT * 8 + lo * 8:
                                     b * KT * 8 + hi * 8],
                    num_idxs=(hi - lo) * 128, num_idxs_reg=(hi - lo) * 128,
                    elem_size=elem, single_packet=False,
                    queue_num=b % 2 if queue is None else queue)

    with tile.TileContext(nc) as tc:
        with (
            tc.tile_pool(name="dram", bufs=1, space="DRAM") as dram,
            tc.tile_pool(name="const", bufs=1) as const,
        ):
            # resident constants
            w1_sb = const.tile([128, FK, H], cdt)
            nc.sync.dma_start(out=w1_sb[:], in_=w1_in[:])
            rw1_sb = const.tile([128, FK, H], cdt)
            nc.sync.dma_start(out=rw1_sb[:], in_=rw1_in[:])
            w2_sb = const.tile([128, HC, OUT], cdt)
            nc.sync.dma_start(out=w2_sb[:], in_=w2_in[:])
            rw2b_sb = const.tile([128, HC + 1, OUT], cdt)
            nc.sync.dma_start(out=rw2b_sb[:], in_=rw2b_in[:])
            bias1_sb = const.tile([128, HC], f32)
            nc.sync.dma_start(out=bias1_sb[:], in_=bias1_in[:])
            idx16_sb = const.tile([128, B * KT * 8], mybir.dt.int16)
            nc.sync.dma_start(out=idx16_sb[:], in_=idx16_in[:])
            idx16b_sb = const.tile([128, B * KT * 8], mybir.dt.int16)
            nc.sync.dma_start(out=idx16b_sb[:], in_=idx16b_in[:])
            ones_sb = const.tile([128, 128], cdt)
            nc.vector.memset(ones_sb[:], 0.0)
            nc.vector.memset(ones_sb[0:1, :], 1.0)
            # dense-2 (h@rw2 + b2) results, computed in phase A
            outd_sb = const.tile([128, B, 128], f32)

            for rep in range(reps):
                hw_locs = [
                    dram.tile([BC * 128, OUT], hwdt, tag=f"hw_loc{g}",
                              name=f"hw_loc{rep}_{g}")
                    for g in range(nchunk)]
                hw_full = dram.tile([NP, OUT], hwdt,
                                    tag="hw_full", name=f"hw_full{rep}")
                if FP8_HW:
                    hw_pad = dram.tile([NP, 2 * OUT], hwdt, tag="hw_pad",
                                       name=f"hw_pad{rep}")
                # ---------------- phase A: layer 1 + hw + dense2 ------------
                with (
                    tc.tile_pool(name=f"xg_pool{rep}", bufs=7) as xg_pool,
                    tc.tile_pool(name=f"sa_pool{rep}", bufs=3) as sa_pool,
                    tc.tile_pool(name=f"axsb_pool{rep}", bufs=2) as axsb_pool,
                    tc.tile_pool(name=f"hstage_pool{rep}", bufs=3) as hstage_pool,
                    tc.tile_pool(name=f"hwsb_pool{rep}", bufs=3) as hwsb_pool,
                    tc.tile_pool(name=f"ax_psum{rep}", bufs=2,
                                 space="PSUM") as ax_psum,
                    tc.tile_pool(name=f"h_psum{rep}", bufs=2,
                                 space="PSUM") as h_psum,
                    tc.tile_pool(name=f"hw_psum{rep}", bufs=2,
                                 space="PSUM") as hw_psum,
                    tc.tile_pool(name=f"o2_psum{rep}", bufs=2,
                                 space="PSUM") as o2_psum,
                ):
                    for b in range(B):
                        bw = slice(b * 128, (b + 1) * 128)
                        if b % GB == 0:
                            xg = xg_pool.tile([128, GB * KT, F], l1dt,
                                              tag="xg")
                            _gather(xg, x_table, idx16_sb, b,
                                    min(GB, B - b), F, ops=OPS_A)
                        goff = (b % GB) * KT
                        s1_tile = sa_pool.tile([128, KT, 128], l1dt,
                                               tag="s1_sb")
                        nc.scalar.dma_start(out=s1_tile[:],
                                            in_=s1_in[:, b, :])
                        xt_tile = sa_pool.tile([128, FK, 128], cdt,
                                               tag="xt_tile")
                        nc.scalar.dma_start(out=xt_tile[:],
                                            in_=xt_in[:, :, bw])

                        # aggregation in input space: axT[fc] = Xg_chunk.T @ S
                        psum_ax = ax_psum.tile([128, FK, 128], f32,
                                               tag="psum_ax")
                        for fc in range(FK):
                            fs = slice(fc * 128, (fc + 1) * 128)
                            if FP8_L1:
                                for k2 in range(KT // 2):
                                    nc.tensor.matmul(
                                        out=psum_ax[:, fc, :],
                                        lhsT=xg[:, goff + 2 * k2:
                                                goff + 2 * k2 + 2, fs],
                                        rhs=s1_tile[:, 2 * k2:2 * k2 + 2, :],
                                        start=(k2 == 0),
                                        stop=(k2 == KT // 2 - 1),
                                        perf_mode=mybir.MatmulPerfMode.DoubleRow,
                                    )
                            else:
                                for kt in range(KT):
                                    nc.tensor.matmul(
                                        out=psum_ax[:, fc, :],
                                        lhsT=xg[:, goff + kt, fs],
                                        rhs=s1_tile[:, kt, :],
                                        start=(kt == 0),
                                        stop=(kt == KT - 1),
                                    )
                        axT_sb = axsb_pool.tile([128, FK, 128], cdt,
                                                tag="axT_sb")
                        nc.vector.tensor_copy(out=axT_sb[:], in_=psum_ax[:])

                        # dense: hT = relu(W1.T @ axT + RW1.T @ xT + b1c)
                        hT_stage = hstage_pool.tile([128, HC, 128], cdt,
                                                    tag="hT_sb")
                        for half in range(2):
                            psum_h = h_psum.tile([128, HC // 2, 128], f32,
                                                 tag="psum_h")
                            for j in range(HC // 2):
                                hc = half * (HC // 2) + j
                                hs = slice(hc * 128, (hc + 1) * 128)
                                for k in range(FK):
                                    nc.tensor.matmul(
                                        out=psum_h[:, j, :],
                                        lhsT=w1_sb[:, k, hs],
                                        rhs=axT_sb[:, k, :],
                                        start=(k == 0), stop=False)
                                for k in range(FK):
                                    nc.tensor.matmul(
                                        out=psum_h[:, j, :],
                                        lhsT=rw1_sb[:, k, hs],
                                        rhs=xt_tile[:, k, :],
                                        start=False, stop=(k == FK - 1))
                            hw0 = half * (HC // 2)
                            for j in range(HC // 2):
                                hc = hw0 + j
                                nc.scalar.activation(
                                    out=hT_stage[:, hc, :],
                                    in_=psum_h[:, j, :],
                                    func=mybir.ActivationFunctionType.Relu,
                                    bias=bias1_sb[:, hc:hc + 1], scale=1.0)

                        # hw = h @ W2   (node-major), stored fp8 for the halo
                        psum_hw = hw_psum.tile([128, OC, 128], f32,
                                               tag="psum_hw")
                        for oc in range(OC):
                            for hc in range(HC):
                                nc.tensor.matmul(
                                    out=psum_hw[:, oc, :],
                                    lhsT=hT_stage[:, hc, :],
                                    rhs=w2_sb[:, hc, oc * 128:(oc + 1) * 128],
                                    start=(hc == 0), stop=(hc == HC - 1))
                        hw_sb = hwsb_pool.tile([128, OUT], hwdt, tag="hw_sb")
                        nc.vector.tensor_copy(out=hw_sb[:], in_=psum_hw[:])
                        lw = slice((b % BC) * 128, (b % BC + 1) * 128)
                        nc.scalar.dma_start(out=hw_locs[b // BC][lw, :],
                                            in_=hw_sb[:])

                        # dense2: outd = h @ rw2 + b2c (phase B adds agg2)
                        psum_o2 = o2_psum.tile([128, OC, 128], f32,
                                               tag="psum_o2")
                        for oc in range(OC):
                            ow = slice(oc * 128, (oc + 1) * 128)
                            for hc in range(HC):
                                nc.tensor.matmul(
                                    out=psum_o2[:, oc, :],
                                    lhsT=hT_stage[:, hc, :],
                                    rhs=rw2b_sb[:, hc, ow],
                                    start=(hc == 0), stop=False)
                            nc.tensor.matmul(
                                out=psum_o2[:, oc, :],
                                lhsT=ones_sb[:],
                                rhs=rw2b_sb[:, HC, ow],
                                start=False, stop=True)
                        nc.vector.tensor_copy(out=outd_sb[:, b, :],
                                              in_=psum_o2[:])

                # chunked all-gather of hw on the SP stream: each chunk
                # launches as soon as its blocks' hw rows are written, and
                # the SP stream has no other phase-A work to block.
                sp_eng = nc.engines[mybir.EngineType.SP]
                for g in range(nchunk):
                    orows = slice(g * C * BC * 128, (g + 1) * C * BC * 128)
                    if no_collective:
                        nc.scalar.dma_start(
                            out=hw_full[orows, :][0:BC * 128, :],
                            in_=hw_locs[g][:])
                    else:
                        nc.gpsimd.collective_compute(
                            "AllGather",
                            mybir.AluOpType.bypass,
                            replica_groups=[list(range(C))],
                            ins=[hw_locs[g][:].opt()],
                            outs=[hw_full[orows, :].opt()],
                        )
                    if FP8_HW:
                        # expand compact fp8 rows into 256B-padded rows so
                        # the phase-B gather meets the 256B stride rule
                        # (both halves written; the pad half is unused but
                        # keeps the finite-checker happy)
                        sp_eng.dma_start(
                            out=hw_pad[orows, 0:OUT],
                            in_=hw_full[orows, :])
                        sp_eng.dma_start(
                            out=hw_pad[orows, OUT:2 * OUT],
                            in_=hw_full[orows, :])

                # ---------------- phase B: layer 2 ----------------
                with (
                    tc.tile_pool(name=f"sb_pool{rep}", bufs=3) as sb_pool,
                    tc.tile_pool(name=f"hwg_pool{rep}", bufs=2) as hwg_pool,
                    tc.tile_pool(name=f"hwb_pool{rep}", bufs=3) as hwb_pool,
                    tc.tile_pool(name=f"osb_pool{rep}", bufs=3) as osb_pool,
                    tc.tile_pool(name=f"o_psum{rep}", bufs=3,
                                 space="PSUM") as o_psum,
                ):
                    for b in range(B):
                        bw = slice(b * 128, (b + 1) * 128)
                        s2_tile = sb_pool.tile([128, KT, 128], cdt,
                                               tag="s2_sb")
                        nc.sync.dma_start(out=s2_tile[:], in_=s2_in[:, b, :])
                        if b % GB == 0:
                            gw = 2 * OUT if FP8_HW else OUT
                            hwg = hwg_pool.tile([128, GB * KT, gw], hwdt,
                                                tag="hwg")
                            tbl = hw_pad if FP8_HW else hw_full
                            _gather(hwg, tbl, idx16b_sb, b,
                                    min(GB, B - b), gw, ops=1,
                                    queue=(b // GB) % 2)
                        goff = (b % GB) * KT
                        if FP8_HW:
                            # payload = [row v | row v+1]; keep first half,
                            # upconvert fp8 -> bf16 for the matmul
                            hwb = hwb_pool.tile([128, KT, OUT], cdt,
                                                tag="hwb")
                            nc.scalar.activation(
                                out=hwb[:],
                                in_=hwg[:, goff:goff + KT, 0:OUT],
                                func=mybir.ActivationFunctionType.Copy,
                                scale=1.0)
                            rhs_t = hwb
                            roff = 0
                        else:
                            rhs_t = hwg
                            roff = goff

                        psum_o = o_psum.tile([128, OC, 128], f32, tag="psum_o")
                        for oc in range(OC):
                            ow = slice(oc * 128, (oc + 1) * 128)
                            for kt in range(KT):
                                nc.tensor.matmul(
                                    out=psum_o[:, oc, :],
                                    lhsT=s2_tile[:, kt, :],
                                    rhs=rhs_t[:, roff + kt,
                                              oc * 128:(oc + 1) * 128],
                                    start=(kt == 0), stop=(kt == KT - 1))
                        out_sb = osb_pool.tile([128, OUT], f32, tag="out_sb")
                        nc.vector.tensor_tensor(
                            out=out_sb[:], in0=psum_o[:, 0, :],
                            in1=outd_sb[:, b, :],
                            op=mybir.AluOpType.add)
                        nc.sync.dma_start(out=out_d[bw, :], in_=out_sb[:])

    nc.compile()
    return nc


# ----------------------------------------------------------------------------
# entry points
# ----------------------------------------------------------------------------
def _run(inputs, trace=False, compute=None, trace_kwargs=None):
    compute = compute or COMPUTE
    cdt, np_cdt = _DT[compute]
    x = np.asarray(inputs["x"])
    cfg = _cfg_from_shapes(x, np.asarray(inputs["w1"]),
                           np.asarray(inputs["w2"]))
    in_maps, KT = _preprocess(
        x, inputs["edge_index"], inputs["edge_weight"],
        inputs["w1"], inputs["b1"], inputs["w2"], inputs["b2"],
        inputs["rw1"], inputs["rb1"], inputs["rw2"], inputs["rb2"],
        cfg, np_cdt)

    key = (tuple(sorted(cfg.items())), KT, compute)
    nc = _prog_cache.get(key)
    if nc is None:
        nc = _build(cfg, KT, cdt)
        _prog_cache[key] = nc

    res = run_bass_kernel_spmd(
        nc, in_maps, core_ids=list(range(C)), trace=trace,
        **(trace_kwargs or {}))

    NL, NLP = cfg["NL"], cfg["NLP"]
    out = np.concatenate(
        [res.results[c]["out"][:NL] for c in range(C)], axis=0)
    return np.ascontiguousarray(out.astype(np.float32)), res


def kernel(**inputs) -> np.ndarray:
    out, _ = _run(inputs, trace=False)
    return out
